# revision 6
# baseline (speedup 1.0000x reference)
"""Trainium2 Bass kernel for a pre-norm transformer encoder layer (SwiGLU FFN).

Shapes (hardcoded): x [2, 2048, 768], mask [2, 2048, 2048] int32,
wq/wk/wv/wo [768, 768], w1/w3 [3072, 768], w2 [768, 3072], g_attn/g_ffn [768].

Sharding: 8 cores = 2 batch x 4 query-slices of 512 tokens. Each core
computes K/V for its full batch element (replicated within the group of 4)
and attention + FFN for its own 512 tokens. No collectives.

Structure: fused slice-major pipeline. Keys are processed in 4 slices of
512 tokens; the K/V projection matmuls for slice i+1 run on the tensor
engine underneath the softmax exp (scalar engine) of slice i, so the
scalar engine's ~25us/slice of exp work is hidden. Per-query softmax
normalization is deferred until after the attention@V accumulation
(normalize-after-AV), and the wo projection contracts over full K=128 by
packing head pairs on the partition axis. FFN accumulates the w2 output
over f-chunks so only 2 prod tiles are ever live.

On-device layout is feature-major ("transposed"): activations [D, tokens].
All matmuls run in bf16 with fp32 PSUM accumulation.
"""
import os
import sys

for _p in ("/opt/trn_rl_repo", "/root/.axon_site/_ro/trn_rl_repo"):
    if os.path.isdir(_p) and _p not in sys.path:
        sys.path.append(_p)

import numpy as np
import ml_dtypes

import concourse.bacc as bacc
import concourse.tile as tile
from concourse import mybir

F32 = mybir.dt.float32
BF16 = mybir.dt.bfloat16
AF = mybir.ActivationFunctionType

B, S, D, H = 2, 2048, 768, 12
DK = D // H            # 64
F = 4 * D              # 3072
T = 512                # local query tokens per core
NCH = D // 128         # 6 feature chunks
NFC = F // 128         # 24 FFN chunks
NKT = S // 128         # 16 key tiles
NSL = 4                # key slices of 512
NQT = S // T           # 4 query slices per batch element
NP = H // 2            # 6 head pairs
EPS = 1e-5


def build_nc():
    nc = bacc.Bacc("TRN2", target_bir_lowering=False, debug=False, num_devices=8)

    xbfT = nc.dram_tensor("xbfT", [NCH, 128, S], BF16, kind="ExternalInput").ap()
    xlocT = nc.dram_tensor("xlocT", [NCH, 128, T], F32, kind="ExternalInput").ap()
    maskT = nc.dram_tensor("maskT", [128, NKT * T], BF16, kind="ExternalInput").ap()
    wqT = nc.dram_tensor("wqT", [NCH, 128, D], BF16, kind="ExternalInput").ap()
    wkT = nc.dram_tensor("wkT", [NCH, 128, D], BF16, kind="ExternalInput").ap()
    wvT = nc.dram_tensor("wvT", [NCH, 128, D], BF16, kind="ExternalInput").ap()
    woP = nc.dram_tensor("woP", [NP, 128, D], BF16, kind="ExternalInput").ap()
    w1T = nc.dram_tensor("w1T", [NFC, 128, D], BF16, kind="ExternalInput").ap()
    w3T = nc.dram_tensor("w3T", [NFC, 128, D], BF16, kind="ExternalInput").ap()
    w2F = nc.dram_tensor("w2F", [NFC, 128, D], BF16, kind="ExternalInput").ap()
    ones16 = nc.dram_tensor("ones16", [128, 128], BF16, kind="ExternalInput").ap()

    outT = nc.dram_tensor("outT", [NCH, 128, T], F32, kind="ExternalOutput").ap()

    from concourse.hw_specs import get_activation_tables
    set_names = list(get_activation_tables(nc.m.arch).keys())
    NLE_ID = set_names.index("natural_log_exp_and_others")

    with tile.TileContext(nc) as tc:
        # Pin the ln/exp/square table once; everything before the FFN silu
        # stays inside this set so the auto-insertion pass adds nothing.
        ld = mybir.InstLoadActFuncSet(
            name=nc.get_next_instruction_name(), ins=[], outs=[],
            act_func_set_id=NLE_ID)
        nc.scalar.add_instruction(ld)

        with tc.tile_pool(name="glob", bufs=1) as Pg:
            ones16_t = Pg.tile([128, 128], BF16, name="ones16_t")
            nc.sync.dma_start(ones16_t[:], ones16)
            eps_t = Pg.tile([128, 1], F32, name="eps_t")
            nc.vector.memset(eps_t[:], EPS)
            warm_rhs = Pg.tile([128, 512], BF16, name="warm_rhs")
            nc.vector.memset(warm_rhs[:], 0.0)

            # ---------------- persistent SBUF state --------------------
            QT = [Pg.tile([128, T], BF16, name=f"QT{c}") for c in range(NCH)]
            xloc = [Pg.tile([128, T], F32, name=f"xloc{c}") for c in range(NCH)]
            hT = [Pg.tile([128, T], F32, name=f"hT{c}") for c in range(NCH)]
            hnT = [Pg.tile([128, T], BF16, name=f"hnT{c}") for c in range(NCH)]
            # attention accumulators: rows 0..63 values, row 64 prob sums
            attnAcc = [Pg.tile([65, T], BF16, name=f"attnAcc{h}") for h in range(H)]
            # head-pair packed normalized attention (rhs of wo)
            attnP = [Pg.tile([128, T], BF16, name=f"attnP{p}") for p in range(NP)]
            wq_t = [Pg.tile([128, D], BF16, name=f"wq{c}") for c in range(NCH)]
            wk_t = [Pg.tile([128, D], BF16, name=f"wk{c}") for c in range(NCH)]
            wv_t = [Pg.tile([128, D], BF16, name=f"wv{c}") for c in range(NCH)]
            woP_t = [Pg.tile([128, D], BF16, name=f"woP{p}") for p in range(NP)]
            rstd = [Pg.tile([128, T], F32, name=f"rstd{i}") for i in range(NSL)]

            with (
                tc.tile_pool(name="attn", bufs=1) as Pa,
                tc.tile_pool(name="ps", bufs=1, space="PSUM") as PS,
            ):
                def warm_mm(n):
                    """Dense matmuls into a dead psum slot to hold PE at 2.4GHz."""
                    wp = PS.tile([128, 768], F32, tag="v", name="warm_ps")
                    for k in range(n):
                        nc.tensor.matmul(wp[:, 0:512], ones16_t[:], warm_rhs[:],
                                         start=True, stop=True)

                # staged DMA: slice-0 critical path first
                xbf = {}
                for c in range(NCH):
                    t0 = Pa.tile([128, T], BF16, tag=f"xbf{c}", bufs=2,
                                 name=f"xbf0_{c}")
                    nc.sync.dma_start(t0[:], xbfT[c][:, 0:T])
                    xbf[(0, c)] = t0
                for c in range(NCH):
                    nc.sync.dma_start(wq_t[c][:], wqT[c])
                    nc.sync.dma_start(wk_t[c][:], wkT[c])

                sq_tiles = {}
                xn = {}

                def emit_norm(i, split=False):
                    """sq -> ms -> rstd -> xn for slice i (tokens i*T..)."""
                    ps_ms = PS.tile([128, T], F32, tag="small", name=f"ms{i}")
                    for c in range(NCH):
                        sq = Pa.tile([128, T], BF16, tag="sq", bufs=2,
                                     name=f"sq{i}_{c}")
                        if split and c % 2 == 1:
                            nc.vector.tensor_mul(sq[:], xbf[(i, c)][:],
                                                 xbf[(i, c)][:])
                        else:
                            nc.scalar.activation(sq[:], xbf[(i, c)][:], AF.Square)
                        nc.tensor.matmul(ps_ms[:], ones16_t[:], sq[:],
                                         start=(c == 0), stop=(c == NCH - 1))
                    lntmp = Pa.tile([128, T], F32, tag="lntmp", bufs=2,
                                    name=f"ln{i}")
                    nc.scalar.activation(lntmp[:], ps_ms[:], AF.Ln,
                                         bias=eps_t[:], scale=1.0 / D)
                    nc.scalar.activation(rstd[i][:], lntmp[:], AF.Exp, scale=-0.5)
                    for c in range(NCH):
                        t = Pa.tile([128, T], BF16, tag=f"xn{c}", bufs=3,
                                    name=f"xn{i}_{c}")
                        nc.vector.tensor_mul(t[:], xbf[(i, c)][:], rstd[i][:])
                        xn[(i, c)] = t

                def dma_xbf(i):
                    for c in range(NCH):
                        t = Pa.tile([128, T], BF16, tag=f"xbf{c}", bufs=2,
                                    name=f"xbf{i}_{c}")
                        nc.sync.dma_start(t[:], xbfT[c][:, i * T:(i + 1) * T])
                        xbf[(i, c)] = t

                def proj_group(i, do, w_t, dest, evac):
                    """One output chunk of a K-style projection for slice i."""
                    ps = PS.tile([128, T], F32, tag="small", name=f"pk{i}_{do}")
                    for c in range(NCH):
                        nc.tensor.matmul(ps[:], w_t[c][:, do * 128:(do + 1) * 128],
                                         xn[(i, c)][:],
                                         start=(c == 0), stop=(c == NCH - 1))
                    if evac == "act":
                        nc.scalar.copy(dest[:], ps[:])
                    else:
                        nc.vector.tensor_copy(dest[:], ps[:])

                # ---------------- P0: norms + Q + K0 ------------------------
                warm_mm(20)
                emit_norm(0, split=True)
                dma_xbf(1)
                for c in range(NCH):
                    nc.sync.dma_start(xloc[c][:], xlocT[c])
                mask_t = {}
                m0 = Pa.tile([128, 4 * T], BF16, tag="mask", bufs=2, name="mask0")
                nc.sync.dma_start(m0[:], maskT[:, 0:4 * T])
                mask_t[0] = m0
                for c in range(NCH):
                    nc.sync.dma_start(wv_t[c][:], wvT[c])
                for p in range(NP):
                    nc.sync.dma_start(woP_t[p][:], woP[p])

                for do in range(NCH):
                    proj_group(0, do, wq_t, QT[do], "act" if do % 2 else "dve")
                KT = {}

                def k_group(i, do):
                    t = Pa.tile([128, T], BF16, tag=f"kt{do}", bufs=2,
                                name=f"KT{i}_{do}")
                    proj_group(i, do, wk_t, t, "act" if do % 2 else "dve")
                    KT[(i, do)] = t

                for do in range(NCH):
                    k_group(0, do)
                emit_norm(1)
                dma_xbf(2)
                emit_norm(2)
                dma_xbf(3)

                # ---------------- P1: fused attention pipeline --------------
                pr = {}
                VA = {}

                def sc_group(i, g, pc):
                    """Scores+exp+mask for heads (2pc, 2pc+1), kt pair g of
                    slice i. Adjacent matmuls alternate row groups 0/64."""
                    h0, h1 = 2 * pc, 2 * pc + 1
                    sc0 = PS.tile([128, 1024], F32, tag="sc", name=f"sc{i}{g}{h0}")
                    sc1 = PS.tile([128, 1024], F32, tag="sc", name=f"sc{i}{g}{h1}")
                    for j in range(2):
                        kt = 2 * g + j
                        ksl = slice(kt * 128, (kt + 1) * 128)
                        nc.tensor.matmul(sc0[:, j * T:(j + 1) * T],
                                         KT[(i, pc)][0:DK, ksl],
                                         QT[pc][0:DK, :], start=True, stop=True)
                        nc.tensor.matmul(sc1[:, j * T:(j + 1) * T],
                                         KT[(i, pc)][DK:128, ksl],
                                         QT[pc][DK:128, :], start=True, stop=True)
                    for h, sc in ((h0, sc0), (h1, sc1)):
                        p = Pa.tile([128, 1024], BF16, tag="pr", bufs=8,
                                    name=f"pr{i}{g}{h}")
                        nc.scalar.activation(p[:], sc[:], AF.Exp)
                        nc.vector.tensor_mul(
                            p[:], p[:], mask_t[i][:, g * 1024:(g + 1) * 1024])
                        pr[(h, g)] = p

                def v_group(i, tt):
                    """Token-major V for token tile tt of slice i, packed with
                    ones columns for the prob-sum row."""
                    ps_v = PS.tile([128, 768], F32, tag="v", name=f"pv{i}_{tt}")
                    tsl = slice(tt * 128, (tt + 1) * 128)
                    for c in range(NCH):
                        nc.tensor.matmul(ps_v[:, 0:512], xn[(i, c)][:, tsl],
                                         wv_t[c][:, 0:512],
                                         start=(c == 0), stop=(c == NCH - 1))
                        nc.tensor.matmul(ps_v[:, 512:768], xn[(i, c)][:, tsl],
                                         wv_t[c][:, 512:768],
                                         start=(c == 0), stop=(c == NCH - 1))
                    va = Pa.tile([128, H * (DK + 1)], BF16, tag="va", bufs=5,
                                 name=f"va{i}_{tt}")
                    ones_cols = va[:].rearrange("p (h e) -> p h e",
                                                e=DK + 1)[:, :, DK:DK + 1]
                    nc.vector.memset(ones_cols, 1.0)
                    nc.vector.tensor_copy(
                        va[:].rearrange("p (h e) -> p h e",
                                        e=DK + 1)[:, :, 0:DK],
                        ps_v[:].rearrange("p (h d) -> p h d", d=DK))
                    VA[(i, tt)] = va

                def av_group(i, h):
                    ps_av = PS.tile([128, T], F32, tag="small", name=f"av{i}_{h}")
                    for kt in range(4):
                        nc.tensor.matmul(
                            ps_av[0:DK + 1, :],
                            VA[(i, kt)][:, h * (DK + 1):(h + 1) * (DK + 1)],
                            pr[(h, kt // 2)][:, (kt % 2) * T:(kt % 2 + 1) * T],
                            start=(kt == 0), stop=(kt == 3))
                    if i == 0:
                        nc.vector.tensor_copy(attnAcc[h][:], ps_av[0:DK + 1, :])
                    else:
                        nc.vector.tensor_add(attnAcc[h][:], ps_av[0:DK + 1, :],
                                             attnAcc[h][:])

                inv_t = {}

                def normalize_pair(p):
                    """1/sums + broadcast + scale, packing the pair into attnP."""
                    for idx, h in enumerate((2 * p, 2 * p + 1)):
                        inv = Pa.tile([1, T], F32, tag="inv", bufs=3,
                                      name=f"inv{h}")
                        nc.vector.reciprocal(inv[:], attnAcc[h][DK:DK + 1, :])
                        bc = Pa.tile([DK, T], F32, tag="bc", bufs=3,
                                     name=f"bc{h}")
                        nc.gpsimd.partition_broadcast(bc[:], inv[:])
                        nc.vector.tensor_mul(
                            attnP[p][idx * DK:(idx + 1) * DK, :],
                            attnAcc[h][0:DK, :], bc[:])

                # V for slice 0 computed in P0's tail
                for tt in range(4):
                    v_group(0, tt)

                for i in range(NSL):
                    if i < NSL - 1:
                        mnext = Pa.tile([128, 4 * T], BF16, tag="mask", bufs=2,
                                        name=f"mask{i + 1}")
                        nc.sync.dma_start(
                            mnext[:], maskT[:, (i + 1) * 4 * T:(i + 2) * 4 * T])
                        mask_t[i + 1] = mnext
                    if i == 0:
                        emit_norm(3)
                    # per head pair: scores (both kt pairs) -> K proj of next
                    # slice (independent, absorbs exp latency) -> attention@V
                    for pc in range(NCH):
                        sc_group(i, 0, pc)
                        sc_group(i, 1, pc)
                        if i < NSL - 1:
                            k_group(i + 1, pc)
                        av_group(i, 2 * pc)
                        av_group(i, 2 * pc + 1)
                        if i == NSL - 1:
                            normalize_pair(pc)
                    # V projection for the next slice under this slice's exps
                    if i < NSL - 1:
                        for tt in range(4):
                            v_group(i + 1, tt)

                # ---------------- P2: wo + residual -------------------------
                warm_mm(16)
                for do in range(NCH):
                    ps_h2 = PS.tile([128, T], F32, tag="small", name=f"h2_{do}")
                    for p in range(NP):
                        nc.tensor.matmul(ps_h2[:],
                                         woP_t[p][:, do * 128:(do + 1) * 128],
                                         attnP[p][:],
                                         start=(p == 0), stop=(p == NP - 1))
                    nc.vector.tensor_add(hT[do][:], ps_h2[:], xloc[do][:])

            # ---------------- P3: FFN ----------------------------------
            with (
                tc.tile_pool(name="ffn", bufs=1) as Pf,
                tc.tile_pool(name="ps2", bufs=1, space="PSUM") as PS2,
            ):
                def warm_mm2(n):
                    wp = PS2.tile([128, T], F32, tag="w", name="warm2_ps")
                    for k in range(n):
                        nc.tensor.matmul(wp[:], ones16_t[:], warm_rhs[:],
                                         start=True, stop=True)

                warm_mm2(24)
                ps_ms2 = PS2.tile([128, T], F32, tag="u", name="ps_ms2")
                for do in range(NCH):
                    sqh = Pf.tile([128, T], BF16, tag="sqh", bufs=2,
                                  name=f"sqh{do}")
                    nc.scalar.activation(sqh[:], hT[do][:], AF.Square)
                    nc.tensor.matmul(ps_ms2[:], ones16_t[:], sqh[:],
                                     start=(do == 0), stop=(do == NCH - 1))
                lntmp2 = Pf.tile([128, T], F32, name="lntmp2")
                nc.scalar.activation(lntmp2[:], ps_ms2[:], AF.Ln,
                                     bias=eps_t[:], scale=1.0 / D)
                rstd2 = Pf.tile([128, T], F32, name="rstd2")
                nc.scalar.activation(rstd2[:], lntmp2[:], AF.Exp, scale=-0.5)
                for c in range(NCH):
                    nc.vector.tensor_mul(hnT[c][:], hT[c][:], rstd2[:])

                ps_y = [PS2.tile([128, T], F32, tag=f"y{do}", name=f"ps_y{do}")
                        for do in range(NCH)]
                for f in range(NFC):
                    w1_t = Pf.tile([128, D], BF16, tag="w1_t", bufs=4,
                                   name=f"w1_{f}")
                    nc.sync.dma_start(w1_t[:], w1T[f])
                    w3_t = Pf.tile([128, D], BF16, tag="w3_t", bufs=4,
                                   name=f"w3_{f}")
                    nc.sync.dma_start(w3_t[:], w3T[f])
                    w2_t = Pf.tile([128, D], BF16, tag="w2_t", bufs=4,
                                   name=f"w2_{f}")
                    nc.sync.dma_start(w2_t[:], w2F[f])
                    ps_u = PS2.tile([128, T], F32, tag="u", name=f"ps_u{f}")
                    ps_w = PS2.tile([128, T], F32, tag="w", name=f"ps_w{f}")
                    for c in range(NCH):
                        csl = slice(c * 128, (c + 1) * 128)
                        nc.tensor.matmul(ps_u[:], w1_t[:, csl], hnT[c][:],
                                         start=(c == 0), stop=(c == NCH - 1))
                        nc.tensor.matmul(ps_w[:], w3_t[:, csl], hnT[c][:],
                                         start=(c == 0), stop=(c == NCH - 1))
                    silu = Pf.tile([128, T], BF16, tag="silu", bufs=2,
                                   name=f"silu{f}")
                    if os.environ.get("BASS_SIM_SILU") == "1":
                        # CoreSim has no Silu; emulate as u*sigmoid(u)
                        nc.scalar.activation(silu[:], ps_u[:], AF.Sigmoid)
                        nc.vector.tensor_mul(silu[:], silu[:], ps_u[:])
                    else:
                        nc.scalar.activation(silu[:], ps_u[:], AF.Silu)
                    prod = Pf.tile([128, T], BF16, tag="prod", bufs=2,
                                   name=f"prod{f}")
                    nc.vector.tensor_mul(prod[:], silu[:], ps_w[:])
                    for do in range(NCH):
                        nc.tensor.matmul(ps_y[do][:],
                                         w2_t[:, do * 128:(do + 1) * 128],
                                         prod[:],
                                         start=(f == 0), stop=(f == NFC - 1))
                for do in range(NCH):
                    outt = Pf.tile([128, T], F32, tag="outt", bufs=2,
                                   name=f"outt{do}")
                    nc.vector.tensor_add(outt[:], ps_y[do][:], hT[do][:])
                    nc.sync.dma_start(outT[do], outt[:])

    nc.compile()
    return nc


def prep_inputs(x, mask, wq, wk, wv, wo, w1, w2, w3, g_attn, g_ffn):
    """Build the 8 per-core input maps (host-side sharding + layout)."""
    bf = ml_dtypes.bfloat16
    # 1/sqrt(DK) folded into wq
    wqTe = np.ascontiguousarray(
        (wq * (g_attn[None, :] / np.sqrt(DK))).T.reshape(NCH, 128, D)).astype(bf)
    wkTe = np.ascontiguousarray((wk * g_attn[None, :]).T.reshape(NCH, 128, D)).astype(bf)
    wvTe = np.ascontiguousarray((wv * g_attn[None, :]).T.reshape(NCH, 128, D)).astype(bf)
    woPe = np.ascontiguousarray(wo.T.reshape(NP, 128, D)).astype(bf)
    w1Te = np.ascontiguousarray(
        (w1 * g_ffn[None, :]).T.reshape(NCH, 128, NFC, 128)
        .transpose(2, 1, 0, 3).reshape(NFC, 128, D)).astype(bf)
    w3Te = np.ascontiguousarray(
        (w3 * g_ffn[None, :]).T.reshape(NCH, 128, NFC, 128)
        .transpose(2, 1, 0, 3).reshape(NFC, 128, D)).astype(bf)
    w2Fe = np.ascontiguousarray(w2.T.reshape(NFC, 128, D)).astype(bf)
    ones16 = np.ones((128, 128), bf)

    in_maps = []
    for core in range(8):
        b, qt = core // NQT, core % NQT
        # rotate tokens so the local 512-query slice is always quarter 0
        order = (np.arange(S) + qt * T) % S
        xb = x[b][order]                       # [S, D] rotated
        xTe = np.ascontiguousarray(xb.T.reshape(NCH, 128, S))
        xbfTe = xTe.astype(bf)
        xlocTe = np.ascontiguousarray(xTe[:, :, 0:T]).astype(np.float32)
        # maskT[p, kt*T + q] = mask[b, qt*T + q, k] with k = kt*128 + p in
        # ROTATED key order (keys follow the same rotation as tokens).
        msl = mask[b, qt * T:(qt + 1) * T][:, order]     # [T(q), S(k)] rotated
        maskTe = np.ascontiguousarray(
            msl.T.reshape(NKT, 128, T).transpose(1, 0, 2)
            .reshape(128, NKT * T)).astype(bf)
        in_maps.append({
            "xbfT": xbfTe, "xlocT": xlocTe, "maskT": maskTe,
            "wqT": wqTe, "wkT": wkTe, "wvT": wvTe, "woP": woPe,
            "w1T": w1Te, "w3T": w3Te, "w2F": w2Fe,
            "ones16": ones16,
        })
    return in_maps


_NC_CACHE = None


def get_nc():
    global _NC_CACHE
    if _NC_CACHE is None:
        _NC_CACHE = build_nc()
    return _NC_CACHE


def gather_output(results):
    out = np.empty((B, S, D), np.float32)
    for core in range(8):
        b, qt = core // NQT, core % NQT
        o = results[core]["outT"]              # [NCH, 128, T]
        out[b, qt * T:(qt + 1) * T, :] = o.reshape(D, T).T
    return out


def kernel(**inputs):
    from concourse.bass_utils import run_bass_kernel_spmd
    in_maps = prep_inputs(
        np.asarray(inputs["x"]), np.asarray(inputs["mask"]),
        np.asarray(inputs["wq"]), np.asarray(inputs["wk"]),
        np.asarray(inputs["wv"]), np.asarray(inputs["wo"]),
        np.asarray(inputs["w1"]), np.asarray(inputs["w2"]),
        np.asarray(inputs["w3"]),
        np.asarray(inputs["g_attn"]), np.asarray(inputs["g_ffn"]))
    nc = get_nc()
    res = run_bass_kernel_spmd(nc, in_maps, core_ids=list(range(8)))
    return gather_output(res.results)
